# revision 1
# baseline (speedup 1.0000x reference)
"""ArcFace (AngularPenaltySMLoss) fused loss kernel for 8 Trainium2 NeuronCores.

Strategy: data-parallel over rows N (each core owns N/8 = 1024 rows of x and
target, streams the full W). Per core, fully fused on-chip:
  1. matmul runs on RAW x (bf16): x^T built by PE transposes right after load;
     the 1/||x|| row normalization is folded into the exp activation's
     per-partition scale AP (exp(S/||x_p|| * psum)), keeping the norm
     computation off the critical path.
  2. stream W in 2048-column tiles: SWDGE load with inline f32->bf16 cast is
     replaced by HWDGE f32 load + DVE cast to a dc-major bf16 layout -> PE
     128x128 transposes staged through the shared 4-bank PSUM slots (emitted
     mid-round so they hide under the tail j-tiles of the previous round) ->
     bf16 matmul (PSUM f32) -> ACT exp with accum_out row-sums.
     logits never touch HBM; ACT is the pacing engine at ~2.5us/j-tile.
  3. target score t_i = (x_i . W[target_i]) / ||x_i|| via indirect-DMA row
     gather (SWDGE, runs at t=0 on the otherwise-idle Q7) + DVE dot; all its
     ACT work (sqrt, exp, ln) runs after the exp stream so the ACT table set
     switches only at the stream edges.
  4. numerator via cos(acos(t)+M) = t*cosM - sinM*sqrt(1-t^2)  (no arccos)
  5. per-core partial sum of L_i; host combines 8 scalars: loss = -sum/8192
"""

import math

import numpy as np

S = 30.0
MARGIN = 0.3
EPS = 1e-7
N, D, C = 8192, 256, 10000
NCORES = 8
NLOC = N // NCORES  # 1024 rows per core
NJ = NLOC // 128  # 8 row-chunks of 128 partitions
CT = 2048  # class-tile width per main-loop round
NR = math.ceil(C / CT)  # 5 rounds (4*2048 + 1808)
NBLK = CT // 128  # 16 row-blocks of W per round

_CACHE = {}


def _build():
    import concourse.bass as bass
    import concourse.mybir as mybir
    import concourse.tile as tile
    from concourse import bacc
    from concourse.masks import make_identity
    from concourse.tile import add_dep_helper

    f32 = mybir.dt.float32
    bf16 = mybir.dt.bfloat16
    i32 = mybir.dt.int32
    AF = mybir.ActivationFunctionType
    OP = mybir.AluOpType

    nc = bacc.Bacc()
    x_ext = nc.declare_dram_parameter("x", [NLOC, D], f32, isOutput=False)
    w_ext = nc.declare_dram_parameter("w", [C, D], f32, isOutput=False)
    t_ext = nc.declare_dram_parameter("tgt", [NLOC, 1], i32, isOutput=False)
    out_ext = nc.declare_dram_parameter("out", [1, 1], f32, isOutput=True)

    with tile.TileContext(nc) as tc:
        with (
            tc.tile_pool(name="singles", bufs=1) as singles,
            tc.tile_pool(name="wnpool", bufs=3) as wnpool,
            tc.tile_pool(name="wbpool", bufs=3) as wbpool,
            tc.tile_pool(name="wtpool", bufs=2) as wtpool,
            tc.tile_pool(name="pmain", bufs=2, space="PSUM") as psum_main,
        ):
            ident = singles.tile([128, 128], bf16)
            make_identity(nc, ident)

            # ------------- phase 0: loads; gathers own the idle Q7 -------------
            xt = singles.tile([128, NJ, D], f32)
            nc.sync.dma_start(
                out=xt, in_=x_ext[:, :].rearrange("(j p) d -> p j d", p=128)
            )
            tg = singles.tile([128, NJ], i32)
            nc.sync.dma_start(
                out=tg, in_=t_ext[:, :].rearrange("(j p) o -> p (j o)", p=128)
            )

            wnb_tiles = [None] * NR
            wt_tiles = [None] * NR

            wn_tiles = [None] * NR

            def load_round(r):
                c0 = r * CT
                cw = min(CT, C - c0)
                nfull = cw // 128
                rem = cw - nfull * 128
                wn = wnpool.tile([128, NBLK, D], f32, tag="wn", name=f"wn{r}")
                if nfull > 0:
                    nc.sync.dma_start(
                        out=wn[:, :nfull, :],
                        in_=w_ext[c0 : c0 + nfull * 128, :].rearrange(
                            "(a p) d -> p a d", p=128
                        ),
                    )
                if rem > 0:
                    nc.sync.dma_start(
                        out=wn[0:rem, nfull, :],
                        in_=w_ext[c0 + nfull * 128 : c0 + cw, :],
                    )
                wn_tiles[r] = wn

            def cast_round(r):
                cw = min(CT, C - r * CT)
                nfull = cw // 128
                rem = cw - nfull * 128
                wn = wn_tiles[r]
                wnb = wbpool.tile(
                    [128, 2, NBLK, 128], bf16, tag="wnb", name=f"wnb{r}"
                )
                if nfull > 0:
                    nc.vector.tensor_copy(
                        out=wnb[:, :, :nfull, :].rearrange("p dc a q -> p a dc q"),
                        in_=wn[:, :nfull, :].rearrange(
                            "p a (dc q) -> p a dc q", dc=2
                        ),
                    )
                if rem > 0:
                    nc.vector.tensor_copy(
                        out=wnb[0:rem, :, nfull, :],
                        in_=wn[0:rem, nfull, :].rearrange("r (dc q) -> r dc q", dc=2),
                    )
                wnb_tiles[r] = wnb

            def stage_round(r):
                # W^T via PE transposes staged through a main-pool PSUM slot
                cw = min(CT, C - r * CT)
                nblk = math.ceil(cw / 128)
                wnb = wnb_tiles[r]
                wt = wtpool.tile(
                    [128, 2, NBLK, 128], bf16, tag="wt"
                )
                ptr_ = psum_main.tile(
                    [128, 2, NBLK, 128], bf16, tag="pm"
                )
                for dc in range(2):
                    for a in range(nblk):
                        rows_a = min(128, cw - a * 128)
                        nc.tensor.transpose(
                            out=ptr_[:, dc, a, 0:rows_a],
                            in_=wnb[0:rows_a, dc, a, :],
                            identity=ident[0:rows_a, 0:rows_a],
                        )
                    nc.vector.tensor_copy(
                        out=wt[:, dc, :, :].rearrange("p a q -> p (a q)")[:, :cw],
                        in_=ptr_[:, dc, :, :].rearrange("p a q -> p (a q)")[:, :cw],
                    )
                wt_tiles[r] = wt

            wg = singles.tile([128, NJ, D], f32)
            for j in range(NJ):
                nc.gpsimd.indirect_dma_start(
                    out=wg[:, j, :],
                    out_offset=None,
                    in_=w_ext[:, :],
                    in_offset=bass.IndirectOffsetOnAxis(ap=tg[:, j : j + 1], axis=0),
                )
            load_round(0)
            load_round(1)

            # raw x^T (stationary operand) via PE transposes
            xnb = singles.tile([128, NJ, D], bf16)
            nc.vector.tensor_copy(out=xnb, in_=xt)
            cast_round(0)
            xnT = singles.tile([128, 2, NLOC], bf16)
            ptx = psum_main.tile([128, 2, NJ, 128], bf16, tag="pm")
            for dc in range(2):
                for j in range(NJ):
                    nc.tensor.transpose(
                        out=ptx[:, dc, j, :],
                        in_=xnb[:, j, dc * 128 : (dc + 1) * 128],
                        identity=ident,
                    )
                nc.vector.tensor_copy(
                    out=xnT[:, dc, :],
                    in_=ptx[:, dc, :, :].rearrange("p j q -> p (j q)"),
                )

            # row norms feeding the exp scale
            xsq = singles.tile([128, NJ, D], f32)
            nc.vector.tensor_tensor(out=xsq, in0=xt, in1=xt, op=OP.mult)
            ss = singles.tile([128, NJ], f32)
            nc.vector.tensor_reduce(
                out=ss, in_=xsq, axis=mybir.AxisListType.X, op=OP.add
            )
            nrm = singles.tile([128, NJ], f32)
            nc.scalar.activation(out=nrm, in_=ss, func=AF.Sqrt)
            rinv = singles.tile([128, NJ], f32)
            nc.vector.reciprocal(out=rinv, in_=nrm)
            srinv = singles.tile([128, NJ], f32)
            nc.vector.tensor_scalar(
                out=srinv, in0=rinv, scalar1=S, scalar2=None, op0=OP.mult
            )

            stage_round(0)
            cast_round(1)

            # ------------- phase 2: main loop over class tiles -------------
            acc = singles.tile([128, NJ, 8], f32)
            nc.vector.memset(acc, 0.0)
            expdump = singles.tile([128, CT], bf16)

            for r in range(NR):
                if r + 2 < NR:
                    load_round(r + 2)
                    cast_round(r + 2)
                c0 = r * CT
                cw = min(CT, C - c0)
                wt = wt_tiles[r]

                nsub = math.ceil(cw / 512)
                for j in range(NJ):
                    if j == 5 and r + 1 < NR:
                        stage_round(r + 1)
                    pm = psum_main.tile([128, CT], f32, tag="pm")
                    for dc in range(2):
                        for s_ in range(nsub):
                            sw = min(512, cw - s_ * 512)
                            nc.tensor.matmul(
                                out=pm[:, s_ * 512 : s_ * 512 + sw],
                                lhsT=xnT[:, dc, j * 128 : (j + 1) * 128],
                                rhs=wt[:, dc, :, :].rearrange("p a q -> p (a q)")[
                                    :, s_ * 512 : s_ * 512 + sw
                                ],
                                start=(dc == 0),
                                stop=(dc == 1),
                                skip_group_check=True,
                            )
                    i_last_exp = nc.scalar.activation(
                        out=expdump[:, :cw],
                        in_=pm[:, :cw],
                        func=AF.Exp,
                        scale=srinv[:, j : j + 1],
                        accum_out=acc[:, j, r : r + 1],
                    )

            # ------------- phase 3: target dot + numerator + combine -------------
            # (its ACT ops run after the exp stream; the gathers and most DVE
            # work gap-fill much earlier)
            traw = singles.tile([128, NJ], f32)
            prod = singles.tile([128, NJ, D], f32)
            for j in range(NJ):
                nc.vector.tensor_tensor(
                    out=prod[:, j, :], in0=xt[:, j, :], in1=wg[:, j, :], op=OP.mult
                )
            nc.vector.tensor_reduce(
                out=traw, in_=prod, axis=mybir.AxisListType.X, op=OP.add
            )
            trn = singles.tile([128, NJ], f32)
            nc.vector.tensor_tensor(out=trn, in0=traw, in1=rinv, op=OP.mult)
            tcl = singles.tile([128, NJ], f32)
            nc.vector.tensor_scalar(
                out=tcl,
                in0=trn,
                scalar1=-1.0 + EPS,
                scalar2=1.0 - EPS,
                op0=OP.max,
                op1=OP.min,
            )
            usq = singles.tile([128, NJ], f32)  # 1 - t^2
            t2 = singles.tile([128, NJ], f32)
            nc.vector.tensor_tensor(out=t2, in0=tcl, in1=tcl, op=OP.mult)
            nc.vector.tensor_scalar(
                out=usq, in0=t2, scalar1=-1.0, scalar2=1.0, op0=OP.mult, op1=OP.add
            )
            rt = singles.tile([128, NJ], f32)  # sqrt(1-t^2)
            nc.scalar.activation(out=rt, in_=usq, func=AF.Sqrt)
            numer = singles.tile([128, NJ], f32)
            tcos = singles.tile([128, NJ], f32)
            nc.vector.tensor_scalar(
                out=tcos, in0=tcl, scalar1=S * math.cos(MARGIN), scalar2=None,
                op0=OP.mult,
            )
            rtm = singles.tile([128, NJ], f32)
            nc.vector.tensor_scalar(
                out=rtm, in0=rt, scalar1=-S * math.sin(MARGIN), scalar2=None,
                op0=OP.mult,
            )
            nc.vector.tensor_tensor(out=numer, in0=rtm, in1=tcos, op=OP.add)

            exp_num = singles.tile([128, NJ], f32)
            nc.scalar.activation(out=exp_num, in_=numer, func=AF.Exp)
            exp_st = singles.tile([128, NJ], f32)
            nc.scalar.activation(out=exp_st, in_=tcl, func=AF.Exp, scale=S)
            rowsum = singles.tile([128, NJ], f32)
            nc.vector.tensor_reduce(
                out=rowsum, in_=acc, axis=mybir.AxisListType.X, op=OP.add
            )
            dtmp = singles.tile([128, NJ], f32)
            nc.vector.tensor_tensor(out=dtmp, in0=rowsum, in1=exp_num, op=OP.add)
            denom = singles.tile([128, NJ], f32)
            nc.vector.tensor_tensor(out=denom, in0=dtmp, in1=exp_st, op=OP.subtract)
            logd = singles.tile([128, NJ], f32)
            nc.scalar.activation(out=logd, in_=denom, func=AF.Ln)
            Lt = singles.tile([128, NJ], f32)
            nc.vector.tensor_tensor(out=Lt, in0=numer, in1=logd, op=OP.subtract)
            Lrow = singles.tile([128, 1], f32)
            nc.vector.tensor_reduce(
                out=Lrow, in_=Lt, axis=mybir.AxisListType.X, op=OP.add
            )
            ones = singles.tile([128, 1], f32)
            nc.vector.memset(ones, 1.0)
            psum_s = psum_main.tile([1, 1], f32, tag="pm")
            nc.tensor.matmul(out=psum_s, lhsT=Lrow, rhs=ones, start=True, stop=True)
            Lp = singles.tile([1, 1], f32)
            nc.vector.tensor_copy(out=Lp, in_=psum_s)
            nc.sync.dma_start(out=out_ext[:, :], in_=Lp)

    nc.finalize()  # Bacc.compile(): reg alloc + sync-wait legalization
    return nc


def _get_nc():
    if "nc" not in _CACHE:
        _CACHE["nc"] = _build()
    return _CACHE["nc"]


def kernel(x, W, target):
    from concourse.bass_utils import run_bass_kernel_spmd

    x = np.ascontiguousarray(np.asarray(x), dtype=np.float32)
    W = np.ascontiguousarray(np.asarray(W), dtype=np.float32)
    tgt = np.ascontiguousarray(np.asarray(target).astype(np.int32).reshape(N, 1))

    nc = _get_nc()
    in_maps = [
        {
            "x": x[c * NLOC : (c + 1) * NLOC],
            "w": W,
            "tgt": tgt[c * NLOC : (c + 1) * NLOC],
        }
        for c in range(NCORES)
    ]
    res = run_bass_kernel_spmd(nc, in_maps, core_ids=list(range(NCORES)))
    parts = np.stack([res.results[i]["out"].reshape(()) for i in range(NCORES)])
    total = np.sum(parts, dtype=np.float32)
    return np.float32(-(total / np.float32(N)))



# revision 22
# speedup vs baseline: 1.2733x; 1.2733x over previous
"""ArcFace (AngularPenaltySMLoss) fused loss kernel for 8 Trainium2 NeuronCores.

v5 strategy (data-parallel over rows N; each core owns N/8 = 1024 rows):
  * Host input marshalling: cast x and W to fp8-e4m3 and stage both the
    row-major form (for the target-row gather / norms) and the D-major
    half-split transposed form [128, 2, cols] (plane-separated, as the fp8
    DoubleRow matmul ISA requires): one matmul contracts all of D=256 at
    0.5 cycles per output column.  Per-core DMA is ~3.3 MB, dense.
  * The exp+row-sum over the [1024 x 10000] logits is split across the two
    engines that can read PSUM, per 1024-column unit:
      A: ACT exact exp with fused accum_out (47 units),
      D: DVE Schraudolph exp (f32 PSUM -> int16 bf16-codes, 33 units),
         swept by a DVE bf16 4x-mode tensor_scalar (mult by 1.0) with
         fused accum_out -> per-row partial sums.
    Schraudolph constant tuned for zero mean multiplicative bias; the
    loose loss tolerance (2e-2 on ~19.2) dwarfs the ~1.8% per-element
    scatter, which also averages out across ~10k summed terms.
    (GPSIMD cannot access PSUM and walrus rejects tensor ops on Pool, so
    it only runs the indirect gathers.)
  * Target score t_i = (x_i . W[target_i]) / ||x_i|| via indirect-DMA row
    gather of the fp8 W + per-j DVE tensor_tensor_reduce dots.
  * numerator via cos(acos(t)+M) = t*cosM - sinM*sqrt(1-t^2); sqrt and
    rsqrt are computed as exp(0.5*ln(.)) so ACT stays on the single
    natural_log_exp_and_others table (zero table switches).
  * Per-core partial sum of L_i; host combines 8 scalars.
"""

import math

import numpy as np

S = 30.0
MARGIN = 0.3
EPS = 1e-7
N, D, C = 8192, 256, 10000
NCORES = 8
NLOC = N // NCORES  # 1024 rows per core
NJ = NLOC // 128  # 8 row-chunks of 128 partitions

UW = 1024  # unit width (columns drained per engine instruction)
NU = math.ceil(C / UW)  # 10 units per j-tile (9*1024 + 784)
# engine shares for the exp stream units (tuned against engine rooflines)
N_ACT, N_DVE = 47, 33  # out of NU*NJ = 80 units
USE_DR = True  # DoubleRow perf-mode matmuls

# Schraudolph bf16 exp: code = round(ASCH*x + BSCH) interpreted as bf16
ASCH = 2.0**7 / math.log(2.0)  # 184.6646544
CSCH = 0.057532  # zero-mean bias constant (round-to-nearest convert)
BSCH = 16256.0 - 128.0 * CSCH

_CACHE = {}


def _unit_engines():
    """Weighted round-robin assignment of the NU*NJ units to A/D."""
    counts = {"A": N_ACT, "D": N_DVE}
    total = sum(counts.values())
    acc = {k: 0.0 for k in counts}
    out = []
    for _ in range(total):
        for k in counts:
            acc[k] += counts[k] / total
        pick = max(acc, key=lambda k: acc[k])
        acc[pick] -= 1.0
        out.append(pick)
    return out


def _build():
    import concourse.bass as bass
    import concourse.mybir as mybir
    import concourse.tile as tile
    from concourse import bacc

    f32 = mybir.dt.float32
    bf16 = mybir.dt.bfloat16
    i16 = mybir.dt.int16
    i32 = mybir.dt.int32
    fp8 = mybir.dt.float8e4
    u8 = mybir.dt.uint8
    AF = mybir.ActivationFunctionType
    OP = mybir.AluOpType
    PM = mybir.MatmulPerfMode.DoubleRow

    ENG = _unit_engines()

    nc = bacc.Bacc()
    xb_ext = nc.declare_dram_parameter("xb", [NLOC, D], bf16, isOutput=False)
    xt_ext = nc.declare_dram_parameter("xt", [128, 2, NLOC], u8, isOutput=False)
    wb_ext = nc.declare_dram_parameter("wb", [C, D], bf16, isOutput=False)
    wt_ext = nc.declare_dram_parameter("wt", [128, 2, C], u8, isOutput=False)
    t_ext = nc.declare_dram_parameter("tgt", [NLOC, 1], i32, isOutput=False)
    out_ext = nc.declare_dram_parameter("out", [1, 1], f32, isOutput=True)

    with tile.TileContext(nc) as tc:
        with (
            tc.tile_pool(name="singles", bufs=1) as singles,
            tc.tile_pool(name="dAp", bufs=2) as dAp,
            tc.tile_pool(name="dIp", bufs=3) as dIp,
            tc.tile_pool(name="dBp", bufs=3) as dBp,
            tc.tile_pool(name="pm", bufs=4, space="PSUM") as pm,
        ):
            # ---------------- phase 0: dense loads; gathers on idle Q7 --------
            xw = singles.tile([128, NJ, D], bf16)  # x rows (bf16, for norms/dots)
            nc.sync.dma_start(
                out=xw, in_=xb_ext[:, :].rearrange("(j p) d -> p j d", p=128)
            )
            tg = singles.tile([128, NJ], i32)
            nc.sync.dma_start(
                out=tg, in_=t_ext[:, :].rearrange("(j p) o -> p (j o)", p=128)
            )
            xT_u = singles.tile([128, 2, NLOC], u8)  # x^T half-planes (lhsT)
            xT = xT_u.bitcast(fp8)
            nc.sync.dma_start(out=xT_u, in_=xt_ext[:, :, :])
            wT_u = singles.tile([128, 2, C], u8)  # W^T half-planes (rhs)
            wT = wT_u.bitcast(fp8)
            WCH = 2048
            for r in range(math.ceil(C / WCH)):
                c0 = r * WCH
                cwd = min(WCH, C - c0)
                nc.sync.dma_start(
                    out=wT_u[:, :, c0 : c0 + cwd], in_=wt_ext[:, :, c0 : c0 + cwd]
                )

            wg = singles.tile([128, NJ, D], bf16)  # gathered W[target] rows
            for j in range(NJ):
                nc.gpsimd.indirect_dma_start(
                    out=wg[:, j, :],
                    out_offset=None,
                    in_=wb_ext[:, :],
                    in_offset=bass.IndirectOffsetOnAxis(ap=tg[:, j : j + 1], axis=0),
                )

            # row norms: ss_j = sum_d x^2 (per-j DVE fused dots on fp8)
            ss = singles.tile([128, NJ], f32)
            prod = singles.tile([128, NJ, D], bf16)
            nc.vector.tensor_tensor(out=prod, in0=xw, in1=xw, op=OP.mult)
            nc.vector.tensor_reduce(
                out=ss, in_=prod, axis=mybir.AxisListType.X, op=OP.add
            )
            # rinv = 1/sqrt(ss) = exp(-0.5*ln(ss)); srinv = S*rinv
            lss = singles.tile([128, NJ], f32)
            nc.scalar.activation(out=lss, in_=ss, func=AF.Ln)
            rinv = singles.tile([128, NJ], f32)
            nc.scalar.activation(out=rinv, in_=lss, func=AF.Exp, scale=-0.5)
            srinv = singles.tile([128, NJ], f32)
            nc.vector.tensor_scalar(
                out=srinv, in0=rinv, scalar1=S, scalar2=None, op0=OP.mult
            )
            srinv_a = singles.tile([128, NJ], f32)
            nc.vector.tensor_scalar(
                out=srinv_a, in0=srinv, scalar1=ASCH, scalar2=None, op0=OP.mult
            )

            # target dots: traw_j = x_j . wg_j (fp8 DVE fused dots)
            traw = singles.tile([128, NJ], f32)
            prod2 = singles.tile([128, NJ, D], bf16)
            nc.vector.tensor_tensor(out=prod2, in0=xw, in1=wg, op=OP.mult)
            nc.vector.tensor_reduce(
                out=traw, in_=prod2, axis=mybir.AxisListType.X, op=OP.add
            )

            # ---------------- phase 2: fused matmul + exp stream ----------------
            acc = singles.tile([128, NJ, NU], f32)

            ui = 0
            for k in range(NU):
                c0 = k * UW
                cw = min(UW, C - c0)
                for j in range(NJ):
                    eng = ENG[ui]
                    ui += 1
                    pmt = pm.tile([128, UW], f32, tag="pm")
                    lhs = xT[:, :, 128 * j : 128 * (j + 1)]
                    for s_ in range(math.ceil(cw / 512)):
                        s0 = s_ * 512
                        sw = min(512, cw - s0)
                        if USE_DR:
                            nc.tensor.matmul(
                                out=pmt[:, s0 : s0 + sw],
                                lhsT=lhs,
                                rhs=wT[:, :, c0 + s0 : c0 + s0 + sw],
                                start=True,
                                stop=True,
                                perf_mode=PM,
                                skip_group_check=True,
                            )
                        else:
                            for h in range(2):
                                nc.tensor.matmul(
                                    out=pmt[:, s0 : s0 + sw],
                                    lhsT=lhs[:, h, :],
                                    rhs=wT[:, h, c0 + s0 : c0 + s0 + sw],
                                    start=(h == 0),
                                    stop=(h == 1),
                                    skip_group_check=True,
                                )
                    if eng == "A":
                        adump = dAp.tile([128, UW], bf16, tag="da")
                        nc.scalar.activation(
                            out=adump[:, :cw],
                            in_=pmt[:, :cw],
                            func=AF.Exp,
                            scale=srinv[:, j : j + 1],
                            accum_out=acc[:, j, k : k + 1],
                        )
                    else:
                        idump = dIp.tile([128, UW], i16, tag="di")
                        nc.vector.tensor_scalar(
                            out=idump[:, :cw],
                            in0=pmt[:, :cw],
                            scalar1=srinv_a[:, j : j + 1],
                            scalar2=BSCH,
                            op0=OP.mult,
                            op1=OP.add,
                        )
                        bdump = dBp.tile([128, UW], bf16, tag="db")
                        nc.vector.tensor_scalar(
                            out=bdump[:, :cw],
                            in0=idump[:, :cw].bitcast(bf16),
                            scalar1=1.0,
                            scalar2=None,
                            op0=OP.mult,
                            op1=OP.add,
                            accum_out=acc[:, j, k : k + 1],
                        )

            # ---------------- phase 3: per-row tail + combine ----------------
            rowsum = singles.tile([128, NJ], f32)
            nc.vector.tensor_reduce(
                out=rowsum, in_=acc, axis=mybir.AxisListType.X, op=OP.add
            )
            trn = singles.tile([128, NJ], f32)
            nc.vector.tensor_tensor(out=trn, in0=traw, in1=rinv, op=OP.mult)
            tcl = singles.tile([128, NJ], f32)
            nc.vector.tensor_scalar(
                out=tcl,
                in0=trn,
                scalar1=-1.0 + EPS,
                scalar2=1.0 - EPS,
                op0=OP.max,
                op1=OP.min,
            )
            usq = singles.tile([128, NJ], f32)  # 1 - t^2
            t2 = singles.tile([128, NJ], f32)
            nc.vector.tensor_tensor(out=t2, in0=tcl, in1=tcl, op=OP.mult)
            nc.vector.tensor_scalar(
                out=usq, in0=t2, scalar1=-1.0, scalar2=1.0, op0=OP.mult, op1=OP.add
            )
            # rt = sqrt(1-t^2) = exp(0.5*ln(usq))
            lusq = singles.tile([128, NJ], f32)
            nc.scalar.activation(out=lusq, in_=usq, func=AF.Ln)
            rt = singles.tile([128, NJ], f32)
            nc.scalar.activation(out=rt, in_=lusq, func=AF.Exp, scale=0.5)
            numer = singles.tile([128, NJ], f32)
            tcos = singles.tile([128, NJ], f32)
            nc.vector.tensor_scalar(
                out=tcos, in0=tcl, scalar1=S * math.cos(MARGIN), scalar2=None,
                op0=OP.mult,
            )
            rtm = singles.tile([128, NJ], f32)
            nc.vector.tensor_scalar(
                out=rtm, in0=rt, scalar1=-S * math.sin(MARGIN), scalar2=None,
                op0=OP.mult,
            )
            nc.vector.tensor_tensor(out=numer, in0=rtm, in1=tcos, op=OP.add)

            exp_num = singles.tile([128, NJ], f32)
            nc.scalar.activation(out=exp_num, in_=numer, func=AF.Exp)
            exp_st = singles.tile([128, NJ], f32)
            nc.scalar.activation(out=exp_st, in_=tcl, func=AF.Exp, scale=S)
            dtmp = singles.tile([128, NJ], f32)
            nc.vector.tensor_tensor(out=dtmp, in0=rowsum, in1=exp_num, op=OP.add)
            denom = singles.tile([128, NJ], f32)
            nc.vector.tensor_tensor(out=denom, in0=dtmp, in1=exp_st, op=OP.subtract)
            logd = singles.tile([128, NJ], f32)
            nc.scalar.activation(out=logd, in_=denom, func=AF.Ln)
            Lt = singles.tile([128, NJ], f32)
            nc.vector.tensor_tensor(out=Lt, in0=numer, in1=logd, op=OP.subtract)
            Lrow = singles.tile([128, 1], f32)
            nc.vector.tensor_reduce(
                out=Lrow, in_=Lt, axis=mybir.AxisListType.X, op=OP.add
            )
            ones = singles.tile([128, 1], f32)
            nc.vector.memset(ones, 1.0)
            psum_s = pm.tile([1, 1], f32, tag="pm")
            nc.tensor.matmul(out=psum_s, lhsT=Lrow, rhs=ones, start=True, stop=True)
            Lp = singles.tile([1, 1], f32)
            nc.vector.tensor_copy(out=Lp, in_=psum_s)
            nc.sync.dma_start(out=out_ext[:, :], in_=Lp)

    nc.finalize()
    return nc


def _get_nc():
    if "nc" not in _CACHE:
        _CACHE["nc"] = _build()
    return _CACHE["nc"]


def _in_maps(x, W, target):
    import concourse.mybir as mybir

    import ml_dtypes

    f8 = mybir.dt.np(mybir.dt.float8e4)
    xq = np.ascontiguousarray(np.asarray(x, dtype=np.float32)).astype(f8)  # [N, D]
    wq = np.ascontiguousarray(np.asarray(W, dtype=np.float32)).astype(f8)  # [C, D]
    xb = np.ascontiguousarray(np.asarray(x, dtype=np.float32)).astype(ml_dtypes.bfloat16)
    wb = np.ascontiguousarray(np.asarray(W, dtype=np.float32)).astype(ml_dtypes.bfloat16)
    # half-split transposed planes: arr[k, h, n] = src[n, h*128 + k]
    xtq = np.ascontiguousarray(xq.reshape(N, 2, 128).transpose(2, 1, 0))
    wtq = np.ascontiguousarray(wq.reshape(C, 2, 128).transpose(2, 1, 0))
    tgt = np.ascontiguousarray(np.asarray(target).astype(np.int32).reshape(N, 1))
    return [
        {
            "xb": xb[c * NLOC : (c + 1) * NLOC],
            "xt": np.ascontiguousarray(xtq[:, :, c * NLOC : (c + 1) * NLOC]).view(np.uint8),
            "wb": wb,
            "wt": wtq.view(np.uint8),
            "tgt": tgt[c * NLOC : (c + 1) * NLOC],
        }
        for c in range(NCORES)
    ]


def kernel(x, W, target):
    from concourse.bass_utils import run_bass_kernel_spmd

    nc = _get_nc()
    res = run_bass_kernel_spmd(nc, _in_maps(x, W, target), core_ids=list(range(NCORES)))
    parts = np.stack([res.results[i]["out"].reshape(()) for i in range(NCORES)])
    total = np.sum(parts, dtype=np.float32)
    return np.float32(-(total / np.float32(N)))


# revision 23
# speedup vs baseline: 1.4241x; 1.1184x over previous
"""ArcFace (AngularPenaltySMLoss) fused loss kernel for 8 Trainium2 NeuronCores.

v5 strategy (data-parallel over rows N; each core owns N/8 = 1024 rows):
  * Host input marshalling: cast x and W to fp8-e4m3 and stage both the
    row-major form (for the target-row gather / norms) and the D-major
    half-split transposed form [128, 2, cols] (plane-separated, as the fp8
    DoubleRow matmul ISA requires): one matmul contracts all of D=256 at
    0.5 cycles per output column.  Per-core DMA is ~3.3 MB, dense.
  * The exp+row-sum over the [1024 x 10000] logits is split across the two
    engines that can read PSUM, per 1024-column unit:
      A: ACT exact exp with fused accum_out (47 units),
      D: DVE Schraudolph exp (f32 PSUM -> int16 bf16-codes, 33 units),
         swept by a DVE bf16 4x-mode tensor_scalar (mult by 1.0) with
         fused accum_out -> per-row partial sums.
    Schraudolph constant tuned for zero mean multiplicative bias; the
    loose loss tolerance (2e-2 on ~19.2) dwarfs the ~1.8% per-element
    scatter, which also averages out across ~10k summed terms.
    (GPSIMD cannot access PSUM and walrus rejects tensor ops on Pool, so
    it only runs the indirect gathers.)
  * Target score t_i = (x_i . W[target_i]) / ||x_i|| via indirect-DMA row
    gather of the fp8 W + per-j DVE tensor_tensor_reduce dots.
  * numerator via cos(acos(t)+M) = t*cosM - sinM*sqrt(1-t^2); sqrt and
    rsqrt are computed as exp(0.5*ln(.)) so ACT stays on the single
    natural_log_exp_and_others table (zero table switches).
  * Per-core partial sum of L_i; host combines 8 scalars.
"""

import math

import numpy as np

S = 30.0
MARGIN = 0.3
EPS = 1e-7
N, D, C = 8192, 256, 10000
NCORES = 8
NLOC = N // NCORES  # 1024 rows per core
NJ = NLOC // 128  # 8 row-chunks of 128 partitions

UW = 1024  # unit width (columns drained per engine instruction)
NU = math.ceil(C / UW)  # 10 units per j-tile (9*1024 + 784)
# engine shares for the exp stream units (tuned against engine rooflines)
N_ACT, N_DVE = 55, 25  # out of NU*NJ = 80 units
USE_DR = True  # DoubleRow perf-mode matmuls

# Schraudolph bf16 exp: code = round(ASCH*x + BSCH) interpreted as bf16
ASCH = 2.0**7 / math.log(2.0)  # 184.6646544
CSCH = 0.057532  # zero-mean bias constant (round-to-nearest convert)
BSCH = 16256.0 - 128.0 * CSCH

_CACHE = {}


def _unit_engines():
    """Weighted round-robin assignment of the NU*NJ units to A/D."""
    counts = {"A": N_ACT, "D": N_DVE}
    total = sum(counts.values())
    acc = {k: 0.0 for k in counts}
    out = []
    for _ in range(total):
        for k in counts:
            acc[k] += counts[k] / total
        pick = max(acc, key=lambda k: acc[k])
        acc[pick] -= 1.0
        out.append(pick)
    return out


def _build():
    import concourse.bass as bass
    import concourse.mybir as mybir
    import concourse.tile as tile
    from concourse import bacc

    f32 = mybir.dt.float32
    bf16 = mybir.dt.bfloat16
    i16 = mybir.dt.int16
    i32 = mybir.dt.int32
    fp8 = mybir.dt.float8e4
    u8 = mybir.dt.uint8
    AF = mybir.ActivationFunctionType
    OP = mybir.AluOpType
    PM = mybir.MatmulPerfMode.DoubleRow

    ENG = _unit_engines()

    nc = bacc.Bacc()
    xb_ext = nc.declare_dram_parameter("xb", [NLOC, D], bf16, isOutput=False)
    xt_ext = nc.declare_dram_parameter("xt", [128, 2, NLOC], u8, isOutput=False)
    wb_ext = nc.declare_dram_parameter("wb", [C, D], bf16, isOutput=False)
    wt_ext = nc.declare_dram_parameter("wt", [128, 2, C], u8, isOutput=False)
    t_ext = nc.declare_dram_parameter("tgt", [NLOC, 1], i32, isOutput=False)
    out_ext = nc.declare_dram_parameter("out", [1, 1], f32, isOutput=True)

    with tile.TileContext(nc) as tc:
        with (
            tc.tile_pool(name="singles", bufs=1) as singles,
            tc.tile_pool(name="dAp", bufs=2) as dAp,
            tc.tile_pool(name="dIp", bufs=3) as dIp,
            tc.tile_pool(name="dBp", bufs=3) as dBp,
            tc.tile_pool(name="pm", bufs=4, space="PSUM") as pm,
        ):
            # ---------------- phase 0: dense loads; gathers on idle Q7 --------
            xw = singles.tile([128, NJ, D], bf16)  # x rows (bf16, for norms/dots)
            nc.sync.dma_start(
                out=xw, in_=xb_ext[:, :].rearrange("(j p) d -> p j d", p=128)
            )
            tg = singles.tile([128, NJ], i32)
            nc.sync.dma_start(
                out=tg, in_=t_ext[:, :].rearrange("(j p) o -> p (j o)", p=128)
            )
            xT_u = singles.tile([128, 2, NLOC], u8)  # x^T half-planes (lhsT)
            xT = xT_u.bitcast(fp8)
            nc.sync.dma_start(out=xT_u, in_=xt_ext[:, :, :])
            WCH = 2048
            wT_chunks = []
            for r in range(math.ceil(C / WCH)):
                c0 = r * WCH
                cwd = min(WCH, C - c0)
                wtc_u = singles.tile([128, 2, cwd], u8, name=f"wtc{r}")
                nc.sync.dma_start(
                    out=wtc_u, in_=wt_ext[:, :, c0 : c0 + cwd]
                )
                wT_chunks.append(wtc_u.bitcast(fp8))

            wg = singles.tile([128, NJ, D], bf16)  # gathered W[target] rows
            for j in range(NJ):
                nc.gpsimd.indirect_dma_start(
                    out=wg[:, j, :],
                    out_offset=None,
                    in_=wb_ext[:, :],
                    in_offset=bass.IndirectOffsetOnAxis(ap=tg[:, j : j + 1], axis=0),
                )

            # row norms: ss_j = sum_d x^2 (per-j DVE fused dots on fp8)
            ss = singles.tile([128, NJ], f32)
            prod = singles.tile([128, NJ, D], bf16)
            nc.vector.tensor_tensor(out=prod, in0=xw, in1=xw, op=OP.mult)
            nc.vector.tensor_reduce(
                out=ss, in_=prod, axis=mybir.AxisListType.X, op=OP.add
            )
            # rinv = 1/sqrt(ss) = exp(-0.5*ln(ss)); srinv = S*rinv
            lss = singles.tile([128, NJ], f32)
            nc.scalar.activation(out=lss, in_=ss, func=AF.Ln)
            rinv = singles.tile([128, NJ], f32)
            nc.scalar.activation(out=rinv, in_=lss, func=AF.Exp, scale=-0.5)
            srinv = singles.tile([128, NJ], f32)
            nc.vector.tensor_scalar(
                out=srinv, in0=rinv, scalar1=S, scalar2=None, op0=OP.mult
            )
            srinv_a = singles.tile([128, NJ], f32)
            nc.vector.tensor_scalar(
                out=srinv_a, in0=srinv, scalar1=ASCH, scalar2=None, op0=OP.mult
            )

            # ---------------- phase 2: fused matmul + exp stream ----------------
            acc = singles.tile([128, NJ, NU], f32)

            ui = 0
            for k in range(NU):
                c0 = k * UW
                cw = min(UW, C - c0)
                for j in range(NJ):
                    eng = ENG[ui]
                    ui += 1
                    pmt = pm.tile([128, UW], f32, tag="pm")
                    lhs = xT[:, :, 128 * j : 128 * (j + 1)]
                    for s_ in range(math.ceil(cw / 512)):
                        s0 = s_ * 512
                        sw = min(512, cw - s0)
                        if USE_DR:
                            cg = c0 + s0
                            nc.tensor.matmul(
                                out=pmt[:, s0 : s0 + sw],
                                lhsT=lhs,
                                rhs=wT_chunks[cg // WCH][
                                    :, :, cg % WCH : cg % WCH + sw
                                ],
                                start=True,
                                stop=True,
                                perf_mode=PM,
                                skip_group_check=True,
                            )
                        else:
                            cg = c0 + s0
                            for h in range(2):
                                nc.tensor.matmul(
                                    out=pmt[:, s0 : s0 + sw],
                                    lhsT=lhs[:, h, :],
                                    rhs=wT_chunks[cg // WCH][
                                        :, h, cg % WCH : cg % WCH + sw
                                    ],
                                    start=(h == 0),
                                    stop=(h == 1),
                                    skip_group_check=True,
                                )
                    if eng == "A":
                        adump = dAp.tile([128, UW], bf16, tag="da")
                        nc.scalar.activation(
                            out=adump[:, :cw],
                            in_=pmt[:, :cw],
                            func=AF.Exp,
                            scale=srinv[:, j : j + 1],
                            accum_out=acc[:, j, k : k + 1],
                        )
                    else:
                        idump = dIp.tile([128, UW], i16, tag="di")
                        nc.vector.tensor_scalar(
                            out=idump[:, :cw],
                            in0=pmt[:, :cw],
                            scalar1=srinv_a[:, j : j + 1],
                            scalar2=BSCH,
                            op0=OP.mult,
                            op1=OP.add,
                        )
                        bdump = dBp.tile([128, UW], bf16, tag="db")
                        nc.vector.tensor_scalar(
                            out=bdump[:, :cw],
                            in0=idump[:, :cw].bitcast(bf16),
                            scalar1=1.0,
                            scalar2=None,
                            op0=OP.mult,
                            op1=OP.add,
                            accum_out=acc[:, j, k : k + 1],
                        )

            # ---------------- phase 3: per-row tail + combine ----------------
            # target dots: traw_j = x_j . wg_j (fp8 DVE fused dots)
            traw = singles.tile([128, NJ], f32)
            prod2 = singles.tile([128, NJ, D], bf16)
            nc.vector.tensor_tensor(out=prod2, in0=xw, in1=wg, op=OP.mult)
            nc.vector.tensor_reduce(
                out=traw, in_=prod2, axis=mybir.AxisListType.X, op=OP.add
            )

            rowsum = singles.tile([128, NJ], f32)
            nc.vector.tensor_reduce(
                out=rowsum, in_=acc, axis=mybir.AxisListType.X, op=OP.add
            )
            trn = singles.tile([128, NJ], f32)
            nc.vector.tensor_tensor(out=trn, in0=traw, in1=rinv, op=OP.mult)
            tcl = singles.tile([128, NJ], f32)
            nc.vector.tensor_scalar(
                out=tcl,
                in0=trn,
                scalar1=-1.0 + EPS,
                scalar2=1.0 - EPS,
                op0=OP.max,
                op1=OP.min,
            )
            usq = singles.tile([128, NJ], f32)  # 1 - t^2
            t2 = singles.tile([128, NJ], f32)
            nc.vector.tensor_tensor(out=t2, in0=tcl, in1=tcl, op=OP.mult)
            nc.vector.tensor_scalar(
                out=usq, in0=t2, scalar1=-1.0, scalar2=1.0, op0=OP.mult, op1=OP.add
            )
            # rt = sqrt(1-t^2) = exp(0.5*ln(usq))
            lusq = singles.tile([128, NJ], f32)
            nc.scalar.activation(out=lusq, in_=usq, func=AF.Ln)
            rt = singles.tile([128, NJ], f32)
            nc.scalar.activation(out=rt, in_=lusq, func=AF.Exp, scale=0.5)
            numer = singles.tile([128, NJ], f32)
            tcos = singles.tile([128, NJ], f32)
            nc.vector.tensor_scalar(
                out=tcos, in0=tcl, scalar1=S * math.cos(MARGIN), scalar2=None,
                op0=OP.mult,
            )
            rtm = singles.tile([128, NJ], f32)
            nc.vector.tensor_scalar(
                out=rtm, in0=rt, scalar1=-S * math.sin(MARGIN), scalar2=None,
                op0=OP.mult,
            )
            nc.vector.tensor_tensor(out=numer, in0=rtm, in1=tcos, op=OP.add)

            exp_num = singles.tile([128, NJ], f32)
            nc.scalar.activation(out=exp_num, in_=numer, func=AF.Exp)
            exp_st = singles.tile([128, NJ], f32)
            nc.scalar.activation(out=exp_st, in_=tcl, func=AF.Exp, scale=S)
            dtmp = singles.tile([128, NJ], f32)
            nc.vector.tensor_tensor(out=dtmp, in0=rowsum, in1=exp_num, op=OP.add)
            denom = singles.tile([128, NJ], f32)
            nc.vector.tensor_tensor(out=denom, in0=dtmp, in1=exp_st, op=OP.subtract)
            logd = singles.tile([128, NJ], f32)
            nc.scalar.activation(out=logd, in_=denom, func=AF.Ln)
            Lt = singles.tile([128, NJ], f32)
            nc.vector.tensor_tensor(out=Lt, in0=numer, in1=logd, op=OP.subtract)
            Lrow = singles.tile([128, 1], f32)
            nc.vector.tensor_reduce(
                out=Lrow, in_=Lt, axis=mybir.AxisListType.X, op=OP.add
            )
            ones = singles.tile([128, 1], f32)
            nc.vector.memset(ones, 1.0)
            psum_s = pm.tile([1, 1], f32, tag="pm")
            nc.tensor.matmul(out=psum_s, lhsT=Lrow, rhs=ones, start=True, stop=True)
            Lp = singles.tile([1, 1], f32)
            nc.vector.tensor_copy(out=Lp, in_=psum_s)
            nc.sync.dma_start(out=out_ext[:, :], in_=Lp)

    nc.finalize()
    return nc


def _get_nc():
    if "nc" not in _CACHE:
        _CACHE["nc"] = _build()
    return _CACHE["nc"]


def _in_maps(x, W, target):
    import concourse.mybir as mybir

    import ml_dtypes

    f8 = mybir.dt.np(mybir.dt.float8e4)
    xq = np.ascontiguousarray(np.asarray(x, dtype=np.float32)).astype(f8)  # [N, D]
    wq = np.ascontiguousarray(np.asarray(W, dtype=np.float32)).astype(f8)  # [C, D]
    xb = np.ascontiguousarray(np.asarray(x, dtype=np.float32)).astype(ml_dtypes.bfloat16)
    wb = np.ascontiguousarray(np.asarray(W, dtype=np.float32)).astype(ml_dtypes.bfloat16)
    # half-split transposed planes: arr[k, h, n] = src[n, h*128 + k]
    xtq = np.ascontiguousarray(xq.reshape(N, 2, 128).transpose(2, 1, 0))
    wtq = np.ascontiguousarray(wq.reshape(C, 2, 128).transpose(2, 1, 0))
    tgt = np.ascontiguousarray(np.asarray(target).astype(np.int32).reshape(N, 1))
    return [
        {
            "xb": xb[c * NLOC : (c + 1) * NLOC],
            "xt": np.ascontiguousarray(xtq[:, :, c * NLOC : (c + 1) * NLOC]).view(np.uint8),
            "wb": wb,
            "wt": wtq.view(np.uint8),
            "tgt": tgt[c * NLOC : (c + 1) * NLOC],
        }
        for c in range(NCORES)
    ]


def kernel(x, W, target):
    from concourse.bass_utils import run_bass_kernel_spmd

    nc = _get_nc()
    res = run_bass_kernel_spmd(nc, _in_maps(x, W, target), core_ids=list(range(NCORES)))
    parts = np.stack([res.results[i]["out"].reshape(()) for i in range(NCORES)])
    total = np.sum(parts, dtype=np.float32)
    return np.float32(-(total / np.float32(N)))
